# revision 2
# baseline (speedup 1.0000x reference)
"""HaloAttention Trainium2 kernel — 8 NeuronCores, data-parallel over (batch x 4-block-row strips).

Per-core strip: x[b, :, 32s-3 : 32s+35, :] zero-padded -> [256, 38, 134].
Pipeline per core:
  1. q~T / k~T channel-major projections (heads zero-padded to 32 rows for 32-aligned
     partition bases), x-patch V projections in pixel-major interleaved layout.
  2. Per block (64): QK per head-pair -> S [128, 196] PSUM; ACT exp (scale folded,
     accum_out = softmax denominator); normalize E by 1/den; PE-transpose -> A^T;
     AV (lhsT = V-interleaved [kpix, 32], rhs = A^T) -> O^T gappy PSUM [4h x 32, 64].
  3. Proj with gap-padded weights batched over 8 blocks (N=512), bias, DMA out (bf16).

Dispatch: the first call builds + compiles the Bass module and a cached
jax.jit(shard_map(bass_exec)) wrapper (the same lowering
bass_utils.run_bass_kernel_spmd uses under axon, which rebuilds and re-traces
that wrapper on every call). Donated output buffers are created on-device by a
tiny cached jit instead of uploading host zeros. If the fast path fails for any
reason we fall back to run_bass_kernel_spmd.
"""

import os
import sys
from contextlib import ExitStack

import numpy as np
import ml_dtypes

if "/opt/trn_rl_repo" not in sys.path:
    sys.path.insert(0, "/opt/trn_rl_repo")

import concourse.bass as bass
import concourse.tile as tile
from concourse import bacc
from concourse import mybir
from concourse.masks import make_identity

BF16 = mybir.dt.bfloat16
F32 = mybir.dt.float32

C = 256
HEADS = 16
HD = 16
HALO = 3
PATCH = 14
H = W = 128
B = 2
SCALE = HD ** -0.5

SR = 4                      # block rows per core
HS = SR * 8 + 2 * HALO      # 38
WS = W + 2 * HALO           # 134
NPIX = HS * WS              # 5092
NINT = SR * 8 * W           # 4096
NBC = 16
NB = SR * NBC               # 64
KA, KB = 126, 70            # patch chunks: rows 0..8 (9*14), rows 9..13 (5*14)

N_CORES = 8

_CACHED = {}


def build_kernel(phases=15, reps=1):
    nc = bacc.Bacc("TRN2", target_bir_lowering=False, debug=False,
                   enable_asserts=False, num_devices=8)

    xs_d = nc.dram_tensor("xs", [2, 128, NPIX], BF16, kind="ExternalInput")
    wqk_d = nc.dram_tensor("wqk", [2, 128, 1024], BF16, kind="ExternalInput")
    wv_d = nc.dram_tensor("wv", [2, 128, 288], BF16, kind="ExternalInput")
    wp_d = nc.dram_tensor("wp", [4, 128, 256], BF16, kind="ExternalInput")
    bp_d = nc.dram_tensor("bp", [128, 2], F32, kind="ExternalInput")
    o_d = nc.dram_tensor("o", [2, 128, 4096], BF16, kind="ExternalOutput")

    with tile.TileContext(nc) as tc, ExitStack() as ctx:
        consts = ctx.enter_context(tc.tile_pool(name="consts", bufs=1))
        bigbuf = ctx.enter_context(tc.tile_pool(name="bigbuf", bufs=1))

        ident = consts.tile([128, 128], BF16)
        make_identity(nc, ident)

        xT = [bigbuf.tile([128, NPIX], BF16, tag=f"xT{c}", name=f"xT{c}") for c in range(2)]
        wqk = [bigbuf.tile([128, 1024], BF16, tag=f"wqk{c}", name=f"wqk{c}") for c in range(2)]
        wv = [bigbuf.tile([128, 288], BF16, tag=f"wv{c}", name=f"wv{c}") for c in range(2)]
        wp = [bigbuf.tile([128, 256], BF16, tag=f"wp{k}", name=f"wp{k}") for k in range(4)]
        bp = consts.tile([128, 2], F32)

        for c in range(2):
            nc.sync.dma_start(xT[c][:], xs_d[c])
            nc.sync.dma_start(wqk[c][:], wqk_d[c])
            nc.sync.dma_start(wv[c][:], wv_d[c])
        for k in range(4):
            nc.sync.dma_start(wp[k][:], wp_d[k])
        nc.sync.dma_start(bp[:], bp_d[:])

        qT = [bigbuf.tile([128, NINT], BF16, tag=f"qT{m}", name=f"qT{m}") for m in range(4)]
        kT = [bigbuf.tile([128, NPIX], BF16, tag=f"kT{m}", name=f"kT{m}") for m in range(4)]

        xT3 = [t[:].rearrange("p (a b) -> p a b", b=WS) for t in xT]     # [128,38,134]
        kT3 = [t[:].rearrange("p (a b) -> p a b", b=WS) for t in kT]

        # ---------------- phase 1: q~ / k~ projections ----------------
        # qT stored block-contiguous: col = 64*blk + 8*qr + qc so QK lhsT is a
        # contiguous 64-col slice (matmul weights APs allow only one free dim).
        qT5 = [t[:].rearrange("p (br cb qr qc) -> p br qr cb qc", br=4, cb=16, qr=8)
               for t in qT]
        with tc.tile_pool(name="qkv_ps", bufs=4, space="PSUM") as qkv_ps:
            for m in range(4):  # q~ chunks (interior pixels, 8 tiles of 4 rows)
                for t in range(8):
                    ps = qkv_ps.tile([128, 512], F32, tag="ps", name="ps")
                    for cc in range(2):
                        rhs = xT3[cc][:, HALO + 4 * t:HALO + 4 * t + 4, HALO:HALO + W]
                        nc.tensor.matmul(ps[:], wqk[cc][:, 128 * m:128 * (m + 1)],
                                         rhs, start=(cc == 0), stop=(cc == 1))
                    br, qr0 = (4 * t) // 8, (4 * t) % 8
                    dst = qT5[m][:, br, qr0:qr0 + 4]
                    nc.any.tensor_copy(out=dst,
                                       in_=ps[:].rearrange("p (a b c) -> p a b c",
                                                           a=4, b=16))
            for m in range(4):  # k~ chunks (all strip pixels)
                for t in range(10):
                    n = min(512, NPIX - 512 * t)
                    ps = qkv_ps.tile([128, 512], F32, tag="ps", name="ps")
                    for cc in range(2):
                        nc.tensor.matmul(ps[:, :n],
                                         wqk[cc][:, 128 * (4 + m):128 * (5 + m)],
                                         xT[cc][:, 512 * t:512 * t + n],
                                         start=(cc == 0), stop=(cc == 1))
                    nc.any.tensor_copy(out=kT[m][:, 512 * t:512 * t + n],
                                       in_=ps[:, :n])

        # ---------------- phase 2: attention ----------------
        with tc.tile_pool(name="s_ps", bufs=2, space="PSUM") as s_ps_pool, \
             tc.tile_pool(name="at_ps", bufs=2, space="PSUM") as at_ps_pool, \
             tc.tile_pool(name="vp_ps", bufs=1, space="PSUM") as vp_ps_pool, \
             tc.tile_pool(name="gap_ps", bufs=1, space="PSUM") as gap_ps_pool, \
             tc.tile_pool(name="op_ps", bufs=1, space="PSUM") as op_ps_pool, \
             tc.tile_pool(name="work", bufs=3) as work, \
             tc.tile_pool(name="epool", bufs=10) as epool, \
             tc.tile_pool(name="atpool", bufs=3) as atpool, \
             tc.tile_pool(name="gpool", bufs=2) as gpool:

            gap_sb = None
            for _rep in range(reps):
              for blk in range(NB):
                  r, cb = blk // NBC, blk % NBC
                  g = cb % 8
                  if g == 0:
                      gap_sb = gpool.tile([128, 4, 512], BF16, tag="gap_sb", name="gap_sb")

                  # --- V patch (interleaved 17 cols/head) ---
                  # im2col the x patch (SBUF->SBUF DMA) so the matmul stationary
                  # operand has a single contiguous free dim
                  xp_sb = work.tile([128, 2, 196], BF16, tag="xp", name="xp")
                  for cc in range(2):
                      nc.gpsimd.tensor_copy(
                          out=xp_sb[:, cc, :].rearrange("p (a b) -> p a b", a=PATCH),
                          in_=xT3[cc][:, 8 * r:8 * r + PATCH, 8 * cb:8 * cb + PATCH])
                  vp_ps_a = vp_ps_pool.tile([128, 288], F32, tag="vpa", name="vpa")
                  vp_ps_b = vp_ps_pool.tile([128, 288], F32, tag="vpb", name="vpb")
                  for cc in range(2):
                      nc.tensor.matmul(vp_ps_a[:KA, :], xp_sb[:, cc, :KA], wv[cc][:],
                                       start=(cc == 0), stop=(cc == 1))
                      nc.tensor.matmul(vp_ps_b[:KB, :], xp_sb[:, cc, KA:196], wv[cc][:],
                                       start=(cc == 0), stop=(cc == 1))
                  vp_a = work.tile([128, 288], BF16, tag="vpa_sb", name="vpa_sb")
                  vp_b = work.tile([128, 288], BF16, tag="vpb_sb", name="vpb_sb")
                  nc.any.tensor_copy(out=vp_a[:KA], in_=vp_ps_a[:KA])
                  nc.any.tensor_copy(out=vp_b[:KB], in_=vp_ps_b[:KB])

                  den = work.tile([128, 8], F32, tag="den", name="den")
                  rden = work.tile([128, 8], F32, tag="rden", name="rden")
                  e_tiles = []
                  for p in (range(8) if phases & 1 else []):
                      s_ps = s_ps_pool.tile([128, 196], F32, tag="s", name="s")
                      for i, hh in enumerate((2 * p, 2 * p + 1)):
                          mc, st = hh // 4, 32 * (hh % 4)
                          lq = qT[mc][st:st + 32, 64 * blk:64 * blk + 64]
                          rk = kT3[mc][st:st + 32, 8 * r:8 * r + PATCH,
                                       8 * cb:8 * cb + PATCH]
                          nc.tensor.matmul(s_ps[64 * i:64 * (i + 1), :], lq, rk,
                                           tile_position=(st, 64 * i))
                      e_sb = epool.tile([128, 196], BF16, tag="e", name="e")
                      nc.scalar.activation(e_sb[:], s_ps[:],
                                           mybir.ActivationFunctionType.Exp,
                                           scale=SCALE, accum_out=den[:, p:p + 1])
                      e_tiles.append(e_sb)

                  if phases & 1:
                      nc.vector.reciprocal(rden[:], den[:])

                  gap_ps = gap_ps_pool.tile([128, 4, 64], F32, tag="gap", name="gap")
                  for p in (range(8) if phases & 2 else []):
                      e_sb = e_tiles[p]
                      nc.gpsimd.tensor_scalar_mul(e_sb[:], e_sb[:], rden[:, p:p + 1])
                      at_ps = at_ps_pool.tile([128, 2, 128], BF16, tag="at", name="at")
                      nc.tensor.transpose(at_ps[:KA, 0, :], e_sb[:, :KA], ident[:])
                      nc.tensor.transpose(at_ps[:KB, 1, :], e_sb[:, KA:196], ident[:])
                      at_sb = atpool.tile([128, 2, 128], BF16, tag="at_sb", name="at_sb")
                      nc.any.tensor_copy(out=at_sb[:], in_=at_ps[:])

                      for i, hh in (enumerate((2 * p, 2 * p + 1)) if phases & 4 else []):
                          outp = gap_ps[32 * (hh % 4):32 * (hh % 4) + 32, hh // 4, :]
                          tp = (0, 32 * (hh % 4))
                          nc.tensor.matmul(outp, vp_a[:KA, 17 * hh:17 * hh + 32],
                                           at_sb[:KA, 0, 64 * i:64 * (i + 1)],
                                           start=True, stop=False, tile_position=tp)
                          nc.tensor.matmul(outp, vp_b[:KB, 17 * hh:17 * hh + 32],
                                           at_sb[:KB, 1, 64 * i:64 * (i + 1)],
                                           start=False, stop=True, tile_position=tp)

                  gs5 = gap_sb[:].rearrange("p k (a b c) -> p k a b c", a=8, b=8)
                  if not (phases & 4):
                      nc.vector.memset(gap_ps[:, :, :], 0.0)
                  nc.any.tensor_copy(
                      out=gs5[:, :, :, g, :],
                      in_=gap_ps[:].rearrange("p k (a c) -> p k a c", a=8))

                  if g == 7 and phases & 8:
                      half = (cb // 8)
                      for mc in range(2):
                          op_ps = op_ps_pool.tile([128, 512], F32, tag="op", name="op")
                          for kc in range(4):
                              nc.tensor.matmul(op_ps[:],
                                               wp[kc][:, 128 * mc:128 * (mc + 1)],
                                               gap_sb[:, kc, :],
                                               start=(kc == 0), stop=(kc == 3))
                          out_sb = work.tile([128, 512], BF16, tag="out_sb", name="out_sb")
                          nc.vector.tensor_scalar_add(out_sb[:], op_ps[:],
                                                      bp[:, mc:mc + 1])
                          # out_sb cols are (qr 8, b'' 8, qc 8); dst rows qr,
                          # block-cols half*64 .. half*64+64 contiguous
                          o3 = o_d[mc].rearrange("p (row col) -> p row col", col=W)
                          nc.sync.dma_start(
                              o3[:, 8 * r:8 * r + 8, 64 * half:64 * half + 64],
                              out_sb[:].rearrange("p (a bc) -> p a bc", a=8))

    nc.compile()
    return nc


def _prep_host(x, w_qkv, w_proj, b_proj):
    bf = ml_dtypes.bfloat16
    xp = np.zeros((B, C, H + 2 * HALO, W + 2 * HALO), bf)
    xp[:, :, HALO:HALO + H, HALO:HALO + W] = np.asarray(x)
    wq, wk, wvv = w_qkv[:C], w_qkv[C:2 * C], w_qkv[2 * C:]

    wqk_pad = np.zeros((1024, C), np.float32)
    for h in range(HEADS):
        wqk_pad[32 * h:32 * h + HD] = wq[HD * h:HD * (h + 1)]
        wqk_pad[512 + 32 * h:512 + 32 * h + HD] = wk[HD * h:HD * (h + 1)]
    wqkT = np.ascontiguousarray(wqk_pad.T).astype(bf).reshape(2, 128, 1024)

    wv_int = np.zeros((288, C), np.float32)
    for h in range(HEADS):
        wv_int[17 * h:17 * h + HD] = wvv[HD * h:HD * (h + 1)]
    wvT = np.ascontiguousarray(wv_int.T).astype(bf).reshape(2, 128, 288)

    wp_gap = np.zeros((512, C), np.float32)
    for h in range(HEADS):
        wp_gap[32 * h:32 * h + HD] = w_proj[:, HD * h:HD * (h + 1)].T
    wpT = np.ascontiguousarray(wp_gap).astype(bf).reshape(4, 128, 256)

    bpT = np.ascontiguousarray(np.asarray(b_proj, np.float32).reshape(2, 128).T)

    in_maps = []
    for core in range(N_CORES):
        b, s = core // 4, core % 4
        strip = xp[b, :, 32 * s:32 * s + HS, :]
        xs = np.ascontiguousarray(strip).reshape(2, 128, NPIX)
        in_maps.append({"xs": xs, "wqk": wqkT, "wv": wvT, "wp": wpT, "bp": bpT})
    return in_maps


def _build_runtime():
    """Compile the Bass module once and build a cached PJRT dispatch wrapper.

    This mirrors bass_utils.run_bass_kernel_spmd's axon path
    (bass2jax.run_bass_via_pjrt) exactly, except the jit wrapper survives
    across calls (run_bass_via_pjrt rebuilds and re-traces it per call) and
    the donated output buffers are created on-device instead of being
    uploaded as host zeros.
    """
    import jax
    import jax.numpy as jnp
    from jax.sharding import Mesh, PartitionSpec, NamedSharding
    from jax.experimental.shard_map import shard_map
    from concourse.bass2jax import (_bass_exec_p, install_neuronx_cc_hook,
                                    partition_id_tensor)

    nc = _CACHED["nc"]
    install_neuronx_cc_hook()

    partition_name = nc.partition_id_tensor.name if nc.partition_id_tensor else None
    in_names, out_names, out_avals = [], [], []
    for alloc in nc.m.functions[0].allocations:
        if not isinstance(alloc, mybir.MemoryLocationSet):
            continue
        name = alloc.memorylocations[0].name
        if alloc.kind == "ExternalInput":
            if name != partition_name:
                in_names.append(name)
        elif alloc.kind == "ExternalOutput":
            shape = tuple(alloc.tensor_shape)
            dtype = mybir.dt.np(alloc.dtype)
            out_names.append(name)
            out_avals.append(jax.core.ShapedArray(shape, dtype))
    n_params = len(in_names)
    n_outs = len(out_avals)
    all_in_names = tuple(in_names + out_names +
                         ([partition_name] if partition_name else []))

    devices = jax.devices()[:N_CORES]
    assert len(devices) == N_CORES
    mesh = Mesh(np.asarray(devices), ("core",))

    def _body(*args):
        operands = list(args)
        if partition_name is not None:
            operands.append(partition_id_tensor())
        outs = _bass_exec_p.bind(
            *operands,
            out_avals=tuple(out_avals),
            in_names=all_in_names,
            out_names=tuple(out_names),
            lowering_input_output_aliases=(),
            sim_require_finite=True,
            sim_require_nnan=True,
            nc=nc,
        )
        return tuple(outs)

    sharded = jax.jit(
        shard_map(_body, mesh=mesh,
                  in_specs=(PartitionSpec("core"),) * (n_params + n_outs),
                  out_specs=(PartitionSpec("core"),) * n_outs),
        donate_argnums=tuple(range(n_params, n_params + n_outs)),
        keep_unused=True)

    zsh = tuple(NamedSharding(mesh, PartitionSpec("core")) for _ in range(n_outs))
    mkz = jax.jit(
        lambda: tuple(jnp.zeros((N_CORES * a.shape[0], *a.shape[1:]), a.dtype)
                      for a in out_avals),
        out_shardings=zsh)

    return {"sharded": sharded, "mkz": mkz, "in_names": in_names,
            "out_avals": out_avals, "n_outs": n_outs}


def _run_fast(in_maps):
    rt = _CACHED["rt"]
    in_names = rt["in_names"]
    concat_in = [np.concatenate([in_maps[c][name] for c in range(N_CORES)], axis=0)
                 for name in in_names]
    z = rt["mkz"]()
    out = rt["sharded"](*concat_in, *z)
    o = np.asarray(out[0])  # [(8*2), 128, 4096] bf16
    return o.reshape(N_CORES, 2, 128, 4096)


def _run_fallback(in_maps):
    from concourse.bass_utils import run_bass_kernel_spmd
    res = run_bass_kernel_spmd(_CACHED["nc"], in_maps, core_ids=list(range(N_CORES)))
    return np.stack([np.stack([res.results[c]["o"][0], res.results[c]["o"][1]])
                     for c in range(N_CORES)])


def kernel(x, w_qkv, w_proj, b_proj):
    if "nc" not in _CACHED:
        _CACHED["nc"] = build_kernel()
    in_maps = _prep_host(np.asarray(x), np.asarray(w_qkv),
                         np.asarray(w_proj), np.asarray(b_proj))
    o_all = None
    if _CACHED.get("rt_failed") is None:
        try:
            if "rt" not in _CACHED:
                _CACHED["rt"] = _build_runtime()
            o_all = _run_fast(in_maps)
        except Exception:
            _CACHED["rt_failed"] = True
            o_all = None
    if o_all is None:
        o_all = _run_fallback(in_maps)

    out = np.zeros((B, C, H, W), np.float32)
    for core in range(N_CORES):
        b, s = core // 4, core % 4
        o = np.concatenate([o_all[core][0], o_all[core][1]],
                           axis=0).reshape(C, 32, W).astype(np.float32)
        out[b, :, 32 * s:32 * s + 32, :] = o
    return out
